# revision 1
# baseline (speedup 1.0000x reference)
"""Trainium2 Bass kernel for LoRA linear: y = x @ (W + 2*B@A).T + b.

Full inputs: x (8, 2048, 2048) f32, W (2048, 2048) f32, b (2048,) f32,
B (2048, 16) f32, A (16, 2048) f32.  Output (8, 2048, 2048) f32.

Sharding: data-parallel over the batch dim — core i computes
y[i] = x[i] @ w.T + b with the merged weight w = W + 2*B@A.

Host-side layout prep (sharding/packing only, no math): inputs are
pre-transposed, pre-cast to bf16, and pre-tiled into the exact SBUF
layouts the device wants, so every DMA is 128 fat descriptors (HWDGE
descriptor generation was the load bottleneck at ~3ns/descriptor):
  xp[c, p, t, sc] = x[c*256+sc, t*128+p]   (s-chunk-major tiles)
  Wp[ob, p, t, oc] = W[ob*512+oc, t*128+p] (o-bank-major tiles)
  BTs = 2*B.T (exact power-of-two scale; bf16 values identical to a
  device-side scale), A cast to bf16.

Device schedule (per core), tuned from perfetto traces:
  - all loads on ONE HWDGE ring (sync) in consumption-priority order
    (the order IS the prefetch schedule); stores on the other (scalar).
  - A and 2B.T land in zero-memset [128, D] tiles so the rank-16 delta
    matmuls are full-K=128 matmuls — identical shape to the GEMM MMs
    (K is free on the PE; K=16 stationaries cost ~+100ns transitions).
  - bank-0 delta merges are two-phase (ACT evicts PSUM to a bf16
    staging tile, DVE adds all-bf16 at 2x rate) so the head merge wave
    is split across two engines instead of serialized on the DVE.
  - throwaway warm-up matmuls keep the PE activity monitor from
    re-throttling the clock during the DMA/DVE-paced head (HAM drops
    the PE to 1.2 GHz after ~3.4us of low activity density).
  - main GEMM is ob-major: per output bank, 16 row-tiles of 16
    accumulating [128,128]x[128,512] bf16 matmuls; DVE adds the bias
    during PSUM->SBUF eviction.  Delta matmuls for bank ob+1 are
    spread two-per-group through the second half of pass ob so the PE
    stream never develops idle clusters.
"""

import numpy as np
import ml_dtypes

import concourse.bacc as bacc
import concourse.mybir as mybir
import concourse.tile as tile
from concourse.bass_utils import run_bass_kernel_spmd

N_CORES = 8
BATCH, S, D = 8, 2048, 2048
RANK = 16
SCALE = 2.0  # alpha / rank = 32 / 16
P = 128  # partitions
FREE = 512  # f32 elems per PSUM bank
ND = D // P  # 16 contraction tiles
NS = S // P  # 16 row tiles per core
NO = D // FREE  # 4 output banks
XC = 128  # s-columns per packed x chunk (one GEMM row-tile)
NXC = S // XC  # 16 packed x chunks

F32 = mybir.dt.float32
BF16 = mybir.dt.bfloat16
BF_NP = ml_dtypes.bfloat16

def build_nc():
    nc = bacc.Bacc(
        "TRN2", target_bir_lowering=False, debug=False, num_devices=N_CORES
    )
    xp_d = nc.dram_tensor("xp", [NXC * P, ND * XC], BF16, kind="ExternalInput").ap()
    Wp_d = nc.dram_tensor("Wp", [NO * P, ND * FREE], BF16, kind="ExternalInput").ap()
    b_d = nc.dram_tensor("b", [D], F32, kind="ExternalInput").ap()
    BTs_d = nc.dram_tensor("BTs", [RANK, D], BF16, kind="ExternalInput").ap()
    A_d = nc.dram_tensor("A", [RANK, D], BF16, kind="ExternalInput").ap()
    out_d = nc.dram_tensor("out", [S, D], F32, kind="ExternalOutput").ap()

    with tile.TileContext(nc) as tc:
        with (
            tc.tile_pool(name="singles", bufs=1) as singles,
            tc.tile_pool(name="yout", bufs=4) as ypool,
            tc.tile_pool(name="dpsum", bufs=1, space="PSUM") as dpsum,
            tc.tile_pool(name="gpsum", bufs=4, space="PSUM") as gpsum,
        ):
            # A / 2B.T replicated into four 32-row bands (rows 32g..32g+15,
            # rest zero) so four rank-16 delta matmuls can run concurrently
            # in the four 32-row PE groups via tile_position
            A_sb = singles.tile([P, D], BF16)
            BTs_sb = singles.tile([P, D], BF16)
            bb = singles.tile([P, D], F32)
            jk = singles.tile([P, FREE], BF16)
            # resident operands, chunk-major to match the host packing
            wq = singles.tile([P, NO, ND, FREE], BF16)
            xT = singles.tile([P, NXC, ND, XC], BF16)

            nc.vector.memset(jk[:], 0.0)
            nc.vector.memset(A_sb[:], 0.0)
            nc.vector.memset(BTs_sb[:], 0.0)

            # ---- load schedule (sync ring; program order = drain order)
            for g in range(4):
                nc.sync.dma_start(out=A_sb[32 * g : 32 * g + RANK, :], in_=A_d[:])
                nc.sync.dma_start(
                    out=BTs_sb[32 * g : 32 * g + RANK, :], in_=BTs_d[:]
                )

            def load_wt(ob, dg_lo=0, dg_hi=ND):
                nc.sync.dma_start(
                    out=wq[:, ob, dg_lo:dg_hi, :],
                    in_=Wp_d[
                        ob * P : (ob + 1) * P, dg_lo * FREE : dg_hi * FREE
                    ].rearrange("p (t o) -> p t o", t=dg_hi - dg_lo),
                )

            def load_x(c):
                nc.sync.dma_start(
                    out=xT[:, c, :, :],
                    in_=xp_d[c * P : (c + 1) * P, :].rearrange(
                        "p (t s) -> p t s", t=ND
                    ),
                )

            # Wt bank 0 sub-chunks interleaved with the first x chunks so
            # row-tile 0's merge-chase overlaps the remaining loads
            load_x(0)
            load_wt(0, 0, 4)
            load_wt(0, 4, 8)
            load_x(1)
            load_wt(0, 8, 12)
            load_wt(0, 12, 16)
            nc.sync.dma_start(out=bb[:], in_=b_d[None, :].broadcast_to([P, D]))
            load_x(2)
            load_x(3)
            load_x(4)
            load_x(5)
            load_wt(1)
            for c in range(6, 11):
                load_x(c)
            load_wt(2)
            for c in range(11, 16):
                load_x(c)
            load_wt(3)

            _jn = [0]

            def junk_mm():
                # throwaway matmul: keeps the PE activity monitor warm
                _jn[0] += 1
                jp = gpsum.tile([P, FREE], F32, tag="gp", name=f"jp{_jn[0]}")
                nc.tensor.matmul(jp[:], jk[:, 0:P], jk[:], start=True, stop=True)

            stg = singles.tile([P, ND, FREE], BF16)

            def delta_cluster(ob, dt0, twophase=False):
                # wq[:, ob, dt0+g, :] += A[:, dblk].T @ (2*B.T)[:, ob-bank]
                # four rank-16 (zero-padded to K=32) deltas run concurrently
                # in the four 32-row PE groups, into the four banks of ONE
                # psum tile.  twophase: a single 4-bank-wide ACT eviction to
                # bf16 staging + a single 4-wide DVE bf16 add (fewer per-op
                # overheads than 16 narrow ops, split across two engines).
                dp = dpsum.tile([P, 4, FREE], F32, tag="dp", name=f"dp{ob}_{dt0}")
                for g in range(4):
                    dt = dt0 + g
                    nc.tensor.matmul(
                        dp[:, g, :],
                        A_sb[32 * g : 32 * (g + 1), dt * P : (dt + 1) * P],
                        BTs_sb[32 * g : 32 * (g + 1), ob * FREE : (ob + 1) * FREE],
                        start=True,
                        stop=True,
                        tile_position=(32 * g, 0),
                    )
                sl = wq[:, ob, dt0 : dt0 + 4, :]
                if twophase:
                    nc.scalar.copy(stg[:, dt0 : dt0 + 4, :], dp[:])
                    nc.vector.tensor_add(sl, stg[:, dt0 : dt0 + 4, :], sl)
                else:
                    nc.vector.tensor_add(sl, dp[:], sl)

            # PE warm-up while the first loads land
            for _ in range(6):
                junk_mm()
            # delta+merge for bank 0, junk-padded (merges chase the Wt0
            # sub-chunk DMAs and the ACT/DVE adds; junk keeps the PE
            # dense so HAM stays at full clock)
            for cl in range(4):
                delta_cluster(0, 4 * cl, twophase=True)
                for _ in range(4):
                    junk_mm()
            for _ in range(14):
                junk_mm()

            def lhs(st, dt):
                return xT[:, st, dt, :]

            # ---- main GEMM, ob-major ----
            for ob in range(NO):
                for st in range(NS):
                    if ob == 0 and 1 <= st <= 3:
                        junk_mm()
                        junk_mm()
                    gp = gpsum.tile([P, FREE], F32, tag="gp", name=f"gp{ob}_{st}")
                    for dt in range(ND):
                        nc.tensor.matmul(
                            gp[:],
                            lhs(st, dt),
                            wq[:, ob, dt, :],
                            start=(dt == 0),
                            stop=(dt == ND - 1),
                        )
                    if ob < NO - 1 and st in (8, 10, 12, 14):
                        delta_cluster(ob + 1, 2 * (st - 8), twophase=True)
                    yo = ypool.tile([P, FREE], F32, tag="yo", name=f"yo{ob}_{st}")
                    nc.vector.tensor_add(
                        yo[:], gp[:], bb[:, ob * FREE : (ob + 1) * FREE]
                    )
                    nc.scalar.dma_start(
                        out=out_d[
                            st * P : (st + 1) * P, ob * FREE : (ob + 1) * FREE
                        ],
                        in_=yo[:],
                    )

    nc.compile()
    return nc


_NC_CACHE = None


def _get_nc():
    global _NC_CACHE
    if _NC_CACHE is None:
        _NC_CACHE = build_nc()
    return _NC_CACHE


def make_in_maps(x, W, b, B, A):
    x = np.asarray(x, dtype=np.float32)
    W = np.asarray(W, dtype=np.float32)
    b = np.ascontiguousarray(b, dtype=np.float32)
    B = np.asarray(B, dtype=np.float32)
    A = np.asarray(A, dtype=np.float32)
    # xp[i, c, p, t, sc] = xT[i, t*128+p, c*256+sc] = x[i, c*256+sc, t*128+p]
    xT = np.ascontiguousarray(x.transpose(0, 2, 1)).astype(BF_NP)
    xp = np.ascontiguousarray(
        xT.reshape(BATCH, ND, P, NXC, XC).transpose(0, 3, 2, 1, 4)
    ).reshape(BATCH, NXC * P, ND * XC)
    # Wp[ob, p, t, oc] = W.T[t*128+p, ob*512+oc] = W[ob*512+oc, t*128+p]
    Wt = np.ascontiguousarray(W.T).astype(BF_NP)
    Wp = np.ascontiguousarray(
        Wt.reshape(ND, P, NO, FREE).transpose(2, 1, 0, 3)
    ).reshape(NO * P, ND * FREE)
    BTs = np.ascontiguousarray(SCALE * B.T).astype(BF_NP)
    Ab = A.astype(BF_NP)
    return [
        {"xp": xp[i], "Wp": Wp, "b": b, "BTs": BTs, "A": Ab}
        for i in range(N_CORES)
    ]


def run(inputs, **spmd_kwargs):
    """Run the SPMD kernel; returns (output, BassKernelResults)."""
    nc = _get_nc()
    in_maps = make_in_maps(**inputs)
    res = run_bass_kernel_spmd(nc, in_maps, core_ids=list(range(N_CORES)), **spmd_kwargs)
    out = np.stack([res.results[i]["out"] for i in range(N_CORES)]).astype(np.float32)
    return out, res


def kernel(x, W, b, B, A):
    out, _ = run({"x": x, "W": W, "b": b, "B": B, "A": A})
    return out



# revision 2
# speedup vs baseline: 1.0036x; 1.0036x over previous
"""Trainium2 Bass kernel for LoRA linear: y = x @ (W + 2*B@A).T + b.

Full inputs: x (8, 2048, 2048) f32, W (2048, 2048) f32, b (2048,) f32,
B (2048, 16) f32, A (16, 2048) f32.  Output (8, 2048, 2048) f32.

Sharding: data-parallel over the batch dim — core i computes
y[i] = x[i] @ w.T + b with the merged weight w = W + 2*B@A.

Mixed-precision K-split: of the 16 K-tiles of the contraction, the
first 14 run as bf16 matmuls and the last 2 as one fp8-e4m3 DoubleRow
matmul (K=256, ~0.56x the PE cycles per K element).  Measured max rel
err 1.335e-2 vs the 2e-2 gate (bf16-only was 3.2e-3; nf8=4 sims at
1.98e-2 — too close).  HW exec: 241.4us (bf16 baseline: 259.5us;
bf16 PE roofline alone is 218.5us + ~13us fixed preamble/tail).

Scale plumbing: fp8 operands carry x*16 and w*64, so the fp8 partial
is 1024x the true value.  The bf16 W tiles are pre-scaled by 1024 on
the host (exact exponent shift) so ALL 16 K-tiles accumulate at
1024x in ONE psum bank; eviction is ACT copy with scale 2^-10 (psum
f32 -> sbuf f32) + DVE bias-add (f32+f32 -> bf16 store, half the
output DMA bytes; host upcasts to f32).

Device-side LoRA delta: as in v1 — rank-16 delta matmuls via 4
concurrent 32-row PE groups, two-phase ACT+DVE merge into the bf16
weight tiles (which are 1024-scaled, as is BTs).  The 4 fp8 W K-tiles
are then quantized post-merge by one ACT copy (scale 1/16 -> e4m3)
per output bank.

Layout/packing (host side, no math beyond dtype-cast/scale-shift):
  xp_bf[c, p, t, m] = bf16 x[c*128+m, t*128+p]        t in 0..11
  xp_f8[c, p, g, j, m] = e4m3(16*x[c*128+m, k(g,j)])  k = (12+2g+j)*128+p
  Wp[ob, p, t, o] = bf16 1024*W[ob*512+o, t*128+p]    all 16 t
  BTs = bf16 2048*B.T,  A = bf16 A
"""

import numpy as np
import ml_dtypes

import concourse.bacc as bacc
import concourse.mybir as mybir
import concourse.tile as tile
from concourse.bass_utils import run_bass_kernel_spmd

N_CORES = 8
BATCH, S, D = 8, 2048, 2048
RANK = 16
P = 128  # partitions
FREE = 512  # f32 elems per PSUM bank
ND = D // P  # 16 contraction tiles
NS = S // P  # 16 row tiles per core
NO = D // FREE  # 4 output banks
XC = 128  # s-columns per packed x chunk (one GEMM row-tile)
NXC = S // XC  # 16 packed x chunks

NF8 = 2  # of the 16 K-tiles, the last NF8 run in fp8 DoubleRow
NBF = ND - NF8  # bf16 K-tiles
NDR = NF8 // 2  # DoubleRow matmuls per output tile
SX = 16.0  # fp8 x scale
SWQ = 1.0 / 16.0  # wq (1024-scaled) -> w8 (64-scaled) quantize factor
WSC = 1024.0  # global weight scale (SX * 64)

F32 = mybir.dt.float32
BF16 = mybir.dt.bfloat16
FP8 = mybir.dt.float8e4
BF_NP = ml_dtypes.bfloat16
F8_NP = ml_dtypes.float8_e4m3


def build_nc():
    nc = bacc.Bacc(
        "TRN2", target_bir_lowering=False, debug=False, num_devices=N_CORES
    )
    xbf_d = nc.dram_tensor(
        "xbf", [NXC * P, NBF * XC], BF16, kind="ExternalInput"
    ).ap()
    xf8_d = nc.dram_tensor(
        "xf8", [NXC * P, NF8 * XC], FP8, kind="ExternalInput"
    ).ap()
    Wp_d = nc.dram_tensor("Wp", [NO * P, ND * FREE], BF16, kind="ExternalInput").ap()
    b_d = nc.dram_tensor("b", [D], BF16, kind="ExternalInput").ap()
    # host-baked [128, D]: rows 32g..32g+15 hold the tensor, rest zeros
    BTs_d = nc.dram_tensor("BTs", [P, D], BF16, kind="ExternalInput").ap()
    A_d = nc.dram_tensor("A", [P, D], BF16, kind="ExternalInput").ap()
    out_d = nc.dram_tensor("out", [S, D], BF16, kind="ExternalOutput").ap()

    with tile.TileContext(nc) as tc:
        with (
            tc.tile_pool(name="singles", bufs=1) as singles,
            tc.tile_pool(name="yout", bufs=4) as ypool,
            tc.tile_pool(name="dpsum", bufs=1, space="PSUM") as dpsum,
            tc.tile_pool(name="gpsum", bufs=4, space="PSUM") as gpsum,
        ):
            # A / 2048*B.T replicated into four 32-row bands (rows
            # 32g..32g+15, rest zero) so four rank-16 delta matmuls run
            # concurrently in the four 32-row PE groups via tile_position
            A_sb = singles.tile([P, D], BF16)
            BTs_sb = singles.tile([P, D], BF16)
            bb = singles.tile([P, D], BF16)
            jk = singles.tile([P, FREE], BF16)
            # resident operands, chunk-major to match the host packing
            wq = singles.tile([P, NO, ND, FREE], BF16)  # 1024-scaled merged W
            wq8 = singles.tile([P, NO, NDR, 2, FREE], FP8)  # 64-scaled fp8 W
            xT = singles.tile([P, NXC, NBF, XC], BF16)
            x8T = singles.tile([P, NXC, NDR, 2, XC], FP8)

            nc.vector.memset(jk[:], 0.0)
            # A/BTs replication + zero-padding baked on the host: one fat
            # DMA each, and no 2048-wide DVE memsets gating the loads.

            # ---- load schedule (sync ring; program order = drain order)
            nc.sync.dma_start(out=A_sb[:], in_=A_d[:])
            nc.sync.dma_start(out=BTs_sb[:], in_=BTs_d[:])

            def load_wt(ob, dg_lo=0, dg_hi=ND):
                nc.sync.dma_start(
                    out=wq[:, ob, dg_lo:dg_hi, :],
                    in_=Wp_d[
                        ob * P : (ob + 1) * P, dg_lo * FREE : dg_hi * FREE
                    ].rearrange("p (t o) -> p t o", t=dg_hi - dg_lo),
                )

            def load_x(c):
                nc.sync.dma_start(
                    out=xT[:, c, :, :],
                    in_=xbf_d[c * P : (c + 1) * P, :].rearrange(
                        "p (t s) -> p t s", t=NBF
                    ),
                )

            def load_x8(c0):
                # 4 chunks of fp8 x per DMA (single small loads would
                # fragment the sync queue with 256B-descriptor transfers)
                nc.sync.dma_start(
                    out=x8T[:, c0 : c0 + 4, :, :, :],
                    in_=xf8_d[c0 * P : (c0 + 4) * P, :].rearrange(
                        "(c p) (g j s) -> p c g j s", p=P, g=NDR, j=2
                    ),
                )

            # Wt bank 0 front-loaded (st=0 needs all of it); dt 12..16
            # first: its merge gates the bank-0 fp8 quantize.  Pass 0
            # consumes ALL x chunks, so W banks 1-3 (not needed until
            # ~80/120/170us) load after the x stream — interleaving them
            # starved pass 0 at the ~350GB/s HBM wall (measured).
            load_x(0)
            load_wt(0, 0, 4)
            load_wt(0, 4, 8)
            load_wt(0, 8, 12)
            load_wt(0, 12, 16)
            load_x8(0)
            load_x(1)
            nc.sync.dma_start(out=bb[:], in_=b_d[None, :].broadcast_to([P, D]))
            load_x(2)
            load_x(3)
            load_x8(4)
            load_x(4)
            load_x(5)
            load_x(6)
            load_x(7)
            load_x8(8)
            load_x(8)
            load_x(9)
            load_x(10)
            load_x8(12)
            for c in range(11, 16):
                load_x(c)
            load_wt(1)
            load_wt(2)
            load_wt(3)

            _jn = [0]

            def junk_mm():
                # throwaway matmul: keeps the PE activity monitor warm
                _jn[0] += 1
                jp = gpsum.tile([P, FREE], F32, tag="gp", name=f"jp{_jn[0]}")
                nc.tensor.matmul(jp[:, 0:256], jk[:, 0:P], jk[:, 0:256], start=True, stop=True)

            stg = singles.tile([P, ND, FREE], BF16)

            def delta_cluster(ob, dt0, twophase=False):
                # wq[:, ob, dt0+g, :] += A[:, dblk].T @ (2048*B.T)[:, ob-bank]
                # four rank-16 (zero-padded to K=32) deltas run concurrently
                # in the four 32-row PE groups, into the four banks of ONE
                # psum tile.  twophase: a single 4-bank-wide ACT eviction to
                # bf16 staging + a single 4-wide DVE bf16 add (fewer per-op
                # overheads than 16 narrow ops, split across two engines).
                dp = dpsum.tile([P, 4, FREE], F32, tag="dp", name=f"dp{ob}_{dt0}")
                for g in range(4):
                    dt = dt0 + g
                    nc.tensor.matmul(
                        dp[:, g, :],
                        A_sb[32 * g : 32 * (g + 1), dt * P : (dt + 1) * P],
                        BTs_sb[32 * g : 32 * (g + 1), ob * FREE : (ob + 1) * FREE],
                        start=True,
                        stop=True,
                        tile_position=(32 * g, 0),
                    )
                if twophase:
                    # 2-dt ACT/DVE quanta: halves the latency to the first
                    # merged dt pair (the head GEMM chases these merges)
                    for h in (0, 2):
                        sl = wq[:, ob, dt0 + h : dt0 + h + 2, :]
                        nc.scalar.copy(stg[:, dt0 + h : dt0 + h + 2, :], dp[:, h : h + 2, :])
                        nc.vector.tensor_add(sl, stg[:, dt0 + h : dt0 + h + 2, :], sl)
                else:
                    sl = wq[:, ob, dt0 : dt0 + 4, :]
                    nc.vector.tensor_add(sl, dp[:], sl)

            def quantize_bank(ob):
                # wq8[:, ob] = e4m3(wq[:, ob, 12:16, :] / 16)   (64-scaled w)
                nc.scalar.activation(
                    wq8[:, ob, :, :, :],
                    wq[:, ob, NBF:ND, :].rearrange("p (g j) o -> p g j o", g=NDR),
                    mybir.ActivationFunctionType.Copy,
                    scale=SWQ,
                )

            # PE warm-up while the first loads land
            for _ in range(6):
                junk_mm()
            # delta+merge for bank 0, junk-padded (merges chase the Wt0
            # sub-chunk DMAs and the ACT/DVE adds; junk keeps the PE
            # dense so HAM stays at full clock).  dt 12-15 first, then
            # its quantize overlaps the remaining merges.
            for cl in range(4):
                delta_cluster(0, 4 * cl, twophase=True)
                for _ in range(7):
                    junk_mm()
            quantize_bank(0)
            for _ in range(28):
                junk_mm()

            # ---- main GEMM, ob-major ----
            for ob in range(NO):
                for st in range(NS):
                    if ob == 0 and 1 <= st <= 3:
                        for _ in range(4):
                            junk_mm()
                    gp = gpsum.tile([P, FREE], F32, tag="gp", name=f"gp{ob}_{st}")
                    for dt in range(NBF):
                        nc.tensor.matmul(
                            gp[:],
                            xT[:, st, dt, :],
                            wq[:, ob, dt, :],
                            start=(dt == 0),
                            stop=False,
                        )
                    for g in range(NDR):
                        nc.tensor.matmul(
                            gp[:],
                            x8T[:, st, g, :, :],
                            wq8[:, ob, g, :, :],
                            start=False,
                            stop=(g == NDR - 1),
                            perf_mode=mybir.MatmulPerfMode.DoubleRow,
                        )
                    if ob < NO - 1 and st in (8, 10, 12, 14):
                        # dt 12-15 first so its quantize has slack
                        delta_cluster(
                            ob + 1, {8: 12, 10: 0, 12: 4, 14: 8}[st], twophase=True
                        )
                    if ob < NO - 1 and st == 11:
                        quantize_bank(ob + 1)
                    # eviction: ACT rescales 1024x psum -> f32 staging,
                    # DVE adds bias -> bf16 store tile
                    ys = ypool.tile([P, FREE], F32, tag="ys", name=f"ys{ob}_{st}")
                    nc.scalar.activation(
                        ys[:],
                        gp[:],
                        mybir.ActivationFunctionType.Copy,
                        scale=1.0 / WSC,
                    )
                    yo = ypool.tile([P, FREE], BF16, tag="yo", name=f"yo{ob}_{st}")
                    nc.vector.tensor_add(
                        yo[:], ys[:], bb[:, ob * FREE : (ob + 1) * FREE]
                    )
                    nc.scalar.dma_start(
                        out=out_d[
                            st * P : (st + 1) * P, ob * FREE : (ob + 1) * FREE
                        ],
                        in_=yo[:],
                    )

    nc.compile()
    return nc


_NC_CACHE = None


def _get_nc():
    global _NC_CACHE
    if _NC_CACHE is None:
        _NC_CACHE = build_nc()
    return _NC_CACHE


def make_in_maps(x, W, b, B, A):
    x = np.asarray(x, dtype=np.float32)
    W = np.asarray(W, dtype=np.float32)
    b = np.ascontiguousarray(b.astype(np.float32), dtype=BF_NP)
    B = np.asarray(B, dtype=np.float32)
    A = np.asarray(A, dtype=np.float32)
    # xT[i, d, s] = x[i, s, d]; chunk c covers s-cols c*128..c*128+127
    xT = np.ascontiguousarray(x.transpose(0, 2, 1))  # f32 [B, D, S]
    xr = xT.reshape(BATCH, ND, P, NXC, XC).transpose(0, 3, 2, 1, 4)
    # bf16 part: dt 0..NBF-1
    xbf = np.ascontiguousarray(xr[:, :, :, :NBF, :]).astype(BF_NP).reshape(
        BATCH, NXC * P, NBF * XC
    )
    # fp8 part: dt NBF..15, pairs (NBF+2g, NBF+2g+1)
    xf8 = (
        np.ascontiguousarray(xr[:, :, :, NBF:, :] * SX)
        .astype(F8_NP)
        .reshape(BATCH, NXC * P, NF8 * XC)
    )
    # Wp[ob, p, t, oc] = 1024 * W.T[t*128+p, ob*512+oc]
    Wt = np.ascontiguousarray(W.T * WSC).astype(BF_NP)
    Wp = np.ascontiguousarray(
        Wt.reshape(ND, P, NO, FREE).transpose(2, 1, 0, 3)
    ).reshape(NO * P, ND * FREE)
    # replicate into four 32-row bands (rows 32g..32g+15, rest zero)
    BTs = np.zeros((P, D), dtype=BF_NP)
    Ab = np.zeros((P, D), dtype=BF_NP)
    for g in range(4):
        BTs[32 * g : 32 * g + RANK] = (2.0 * WSC * B.T).astype(BF_NP)
        Ab[32 * g : 32 * g + RANK] = A.astype(BF_NP)
    return [
        {"xbf": xbf[i], "xf8": xf8[i], "Wp": Wp, "b": b, "BTs": BTs, "A": Ab}
        for i in range(N_CORES)
    ]


def run(inputs, **spmd_kwargs):
    """Run the SPMD kernel; returns (output, BassKernelResults)."""
    nc = _get_nc()
    in_maps = make_in_maps(**inputs)
    res = run_bass_kernel_spmd(nc, in_maps, core_ids=list(range(N_CORES)), **spmd_kwargs)
    out = np.stack(
        [np.asarray(res.results[i]["out"]) for i in range(N_CORES)]
    ).astype(np.float32)
    return out, res


def kernel(x, W, b, B, A):
    out, _ = run({"x": x, "W": W, "b": b, "B": B, "A": A})
    return out
